# revision 6
# baseline (speedup 1.0000x reference)
"""Trainium2 Bass kernel for nn_ConvectionModule.

Math (reference):
    s = Z @ W_V                                  # [N]
    A = softmax(sigmoid(s_i - s_j), axis=1)      # [N, N]
    out = A @ (Z @ W_C.T)                        # [N, D]

Low-rank formulation used here:
    E[i, j] = g(s_i - s_j),  g(x) = exp(sigmoid(x)),  is a smooth analytic
    kernel of the bounded scalar difference s_i - s_j, hence numerically
    low-rank.  Chebyshev interpolation in the s_i variable on R nodes x_k:

        E[i, j] ~= sum_k L_k(s_i) * g(x_k - s_j)  =  (L @ B)[i, j]

    with uniform error ~2e-5 at R=16 (measured for this s distribution; the
    final error is dominated by bf16 input rounding, not the rank).  Row
    sums (softmax denominators) follow from the same factorization:
    den = L @ u with u_k = sum_j B[k, j].  Folding 1/den into A-hat = L/den:

        out = A-hat @ (B @ Z) @ W_C.T

    The N^2 work disappears: the device computes U = B@Z (contraction over
    N, bf16, DMA-bound), folds W_C once into U-tilde = U @ W_C.T, and then
    each 128-row output tile is a single rank-16 matmul  y = A-hat @ U-tilde.
    That matmul cancels ~10x, so it runs in float32r (TF32-like, ~1.6e-4
    measured on HW) — full PE speed at 512-wide moving dim.

    L, B, A-hat are host-computed from s (O(N*R) work, same spirit as the
    host-computed s/SVT/SIB of the direct formulation).

Sharding: output rows split across 8 cores, 1024 each.  Every core streams
the full [Z | B.T] (8192 x 528 bf16) and differs only in its A-hat block.
"""

import numpy as np

N = 8192
D = 512
R = 16                     # Chebyshev rank
NCORES = 8
M = N // NCORES            # 1024 rows per core
P = 128
JT = N // P                # 64 j-tiles
GRP = 4                    # j-tiles per DMA
NG = JT // GRP             # 16 DMA groups
ISUB = M // P              # 8 i-subtiles per core

_CACHE = {}


# --------------------------------------------------------------------------
# Kernel build
# --------------------------------------------------------------------------

def _build():
    import concourse.bass as bass  # noqa: F401
    import concourse.mybir as mybir
    import concourse.tile as tile
    from concourse import bacc
    from concourse.masks import make_identity

    f32 = mybir.dt.float32
    f32r = mybir.dt.float32r
    bf16 = mybir.dt.bfloat16

    nc = bacc.Bacc("TRN2", target_bir_lowering=False, debug=False,
                   num_devices=NCORES)

    # ZB = [Z | B.T] bf16; AHT = (L/den).T f32r for this core's row block;
    # WCT = W_C.T bf16.
    ZB = nc.dram_tensor("ZB", [N, D + R], bf16, kind="ExternalInput").ap()
    AHT = nc.dram_tensor("AHT", [R, M], f32r, kind="ExternalInput").ap()
    WCT = nc.dram_tensor("WCT", [D, D], bf16, kind="ExternalInput").ap()
    Y = nc.dram_tensor("Y", [M, D], bf16, kind="ExternalOutput").ap()

    with tile.TileContext(nc) as tc:
        with (
            tc.tile_pool(name="const", bufs=1) as constp,
            tc.tile_pool(name="zb", bufs=NG) as zbp,
            tc.tile_pool(name="fin", bufs=8) as finp,
            tc.tile_pool(name="psU", bufs=1, space="PSUM") as psU,
            tc.tile_pool(name="psT", bufs=1, space="PSUM") as psT,
            tc.tile_pool(name="psW", bufs=1, space="PSUM") as psW,
            tc.tile_pool(name="psO", bufs=5, space="PSUM") as psO,
        ):
            # ---- stream ZB (the only bulk input) ---------------------------
            # last group is split into single-tile DMAs: per-tile semaphores
            # let the final U matmuls start ~0.6us earlier
            zbs = []
            for g in range(NG - 1):
                zb = zbp.tile([P, GRP, D + R], bf16, tag="zb", name=f"zb{g}")
                nc.sync.dma_start(
                    zb[:],
                    ZB[g * GRP * P:(g + 1) * GRP * P, :].rearrange(
                        "(q p) c -> p q c", p=P))
                zbs.append(zb)
            ztail = []
            for q in range(GRP):
                t = (NG - 1) * GRP + q
                zb1 = zbp.tile([P, 1, D + R], bf16, tag="zb", name=f"zbt{q}")
                nc.sync.dma_start(
                    zb1[:], ZB[t * P:(t + 1) * P, :].rearrange(
                        "(q p) c -> p q c", p=P))
                ztail.append(zb1)

            # small tail-phase inputs, after the ZB stream
            wcb = constp.tile([P, 4, D], bf16)
            nc.sync.dma_start(wcb[:], WCT.rearrange("(dc dd) o -> dd dc o",
                                                    dd=P))
            aht = constp.tile([R, M], f32r)
            nc.sync.dma_start(aht[:], AHT)

            # ---- constants -------------------------------------------------
            # warm the ACT copy-function table during the DMA stream (the
            # first scalar.copy otherwise pays a ~1.3us LoadActFuncSet)
            warm = constp.tile([1, 2], f32)
            nc.vector.memset(warm[:], 0.0)
            nc.scalar.copy(warm[:], warm[:])

            id_b = constp.tile([P, P], bf16)
            make_identity(nc, id_b)

            # Warm the PE HAM clock-gate during the startup DMA window:
            # ~3us of dummy matmul activity lifts the PE to 2.4 GHz before
            # the first real matmul issues.
            for w in range(56):
                wp = psT.tile([P, 64], f32, tag="tp", name=f"wp{w}")
                nc.tensor.matmul(wp[:], id_b[:], id_b[:, 0:64],
                                 start=True, stop=True)

            # ---- U = B @ Z : accumulate 64 j-tiles in one PSUM bank --------
            u_ps = psU.tile([R, D], f32, tag="u")
            k = 0
            for g in range(NG - 1):
                for q in range(GRP):
                    nc.tensor.matmul(u_ps[:], zbs[g][:, q, D:D + R],
                                     zbs[g][:, q, 0:D],
                                     start=(k == 0), stop=False)
                    k += 1
            for q in range(GRP):
                nc.tensor.matmul(u_ps[:], ztail[q][:, 0, D:D + R],
                                 ztail[q][:, 0, 0:D],
                                 start=False, stop=(q == GRP - 1))
            u_sb = constp.tile([R, D], bf16)
            nc.vector.tensor_copy(u_sb[:, 0:D // 2], u_ps[:, 0:D // 2])
            nc.vector.tensor_copy(u_sb[:, D // 2:D], u_ps[:, D // 2:D])

            # ---- fold W_C once: U-tilde = U @ W_C.T ------------------------
            ut = constp.tile([P, 4, R], bf16)
            for dc in range(4):
                tp = psT.tile([P, R], bf16, tag="tp")
                nc.tensor.transpose(tp[:], u_sb[:, dc * P:(dc + 1) * P],
                                    id_b[0:R, 0:R])
                nc.vector.tensor_copy(ut[:, dc, :], tp[:])
            ut_ps = psW.tile([R, D], f32, tag="utp")
            for dc in range(4):
                nc.tensor.matmul(ut_ps[:], ut[:, dc, :], wcb[:, dc, :],
                                 start=(dc == 0), stop=(dc == 3))
            ut_r = constp.tile([R, D], f32r)
            nc.vector.tensor_copy(ut_r[:, 0:D // 2], ut_ps[:, 0:D // 2])
            nc.vector.tensor_copy(ut_r[:, D // 2:D], ut_ps[:, D // 2:D])

            # ---- per-row-tile: y = A-hat @ U-tilde (f32r), then out --------
            # pairs of row tiles share one output DMA (the per-DMA trigger
            # cost, not bytes, paces the out phase); each PSUM->SBUF copy is
            # split across ACT and DVE so both engines run in parallel
            for gpair in range(ISUB // 2):
                ysb = finp.tile([P, 2, D], bf16, tag="ysb")
                for h in range(2):
                    s = gpair * 2 + h
                    y_ps = psO.tile([P, D], f32, tag="po")
                    nc.tensor.matmul(y_ps[:], aht[:, s * P:(s + 1) * P],
                                     ut_r[:], start=True, stop=True)
                    nc.scalar.copy(ysb[:, h, 0:D // 2], y_ps[:, 0:D // 2])
                    nc.vector.tensor_copy(ysb[:, h, D // 2:D],
                                          y_ps[:, D // 2:D])
                nc.sync.dma_start(
                    Y[gpair * 2 * P:(gpair + 1) * 2 * P, :].rearrange(
                        "(q p) d -> p q d", p=P),
                    ysb[:])

    nc.compile()
    return nc


# --------------------------------------------------------------------------
# Host-side low-rank factor preparation
# --------------------------------------------------------------------------

def make_in_maps(Z, W_C, W_V):
    import ml_dtypes

    Z = np.ascontiguousarray(Z, dtype=np.float32)
    W_C = np.ascontiguousarray(W_C, dtype=np.float32)
    W_V = np.ascontiguousarray(W_V, dtype=np.float32).reshape(D)

    Zb = Z.astype(ml_dtypes.bfloat16)
    # s on the bf16-rounded Z the device also sees
    s = Zb.astype(np.float64) @ W_V.astype(np.float64)

    # Chebyshev nodes covering the realized s range
    lo, hi = s.min() - 1e-3, s.max() + 1e-3
    kk = np.arange(R)
    theta = (2 * kk + 1) * np.pi / (2 * R)
    xk = 0.5 * (lo + hi) + 0.5 * (hi - lo) * np.cos(theta)

    # B[k, j] = g(x_k - s_j), g = exp(sigmoid)
    B = np.exp(1.0 / (1.0 + np.exp(-(xk[:, None] - s[None, :]))))
    Bb = B.astype(ml_dtypes.bfloat16)

    ZBh = np.zeros((N, D + R), dtype=ml_dtypes.bfloat16)
    ZBh[:, :D] = Zb
    ZBh[:, D:] = Bb.T

    # Lagrange basis at the s_i (barycentric form, fp64)
    w = (-1.0) ** kk * np.sin(theta)
    diff = s[:, None] - xk[None, :]
    hit = np.abs(diff) < 1e-300
    with np.errstate(divide="ignore", invalid="ignore"):
        c = w[None, :] / np.where(hit, 1.0, diff)
    L = c / c.sum(axis=1, keepdims=True)
    if hit.any():
        L[hit.any(axis=1)] = hit[hit.any(axis=1)].astype(np.float64)

    # fold softmax denominators (den = L @ u, u from the bf16 B the device
    # actually contracts with)
    u = Bb.astype(np.float64).sum(axis=1)
    den = L @ u
    ahat = L / den[:, None]

    wct = np.ascontiguousarray(W_C.T).astype(ml_dtypes.bfloat16)

    in_maps = []
    for core in range(NCORES):
        ahT = np.ascontiguousarray(
            ahat[core * M:(core + 1) * M].T.astype(np.float32))
        in_maps.append({"ZB": ZBh, "AHT": ahT, "WCT": wct})
    return in_maps


def kernel(Z, W_C, W_V):
    from concourse.bass_utils import run_bass_kernel_spmd

    if "nc" not in _CACHE:
        _CACHE["nc"] = _build()
    nc = _CACHE["nc"]

    in_maps = make_in_maps(Z, W_C, W_V)
    res = run_bass_kernel_spmd(nc, in_maps, core_ids=list(range(NCORES)))
    out = np.empty((N, D), dtype=np.float32)
    for c in range(NCORES):
        out[c * M:(c + 1) * M] = np.asarray(res.results[c]["Y"],
                                            dtype=np.float32)
    return out


# revision 10
# speedup vs baseline: 1.0254x; 1.0254x over previous
"""Trainium2 Bass kernel for nn_ConvectionModule.

Math (reference):
    s = Z @ W_V                                  # [N]
    A = softmax(sigmoid(s_i - s_j), axis=1)      # [N, N]
    out = A @ (Z @ W_C.T)                        # [N, D]

Low-rank formulation used here:
    E[i, j] = g(s_i - s_j),  g(x) = exp(sigmoid(x)),  is a smooth analytic
    kernel of the bounded scalar difference s_i - s_j, hence numerically
    low-rank.  Chebyshev interpolation in the s_i variable on R nodes x_k:

        E[i, j] ~= sum_k L_k(s_i) * g(x_k - s_j)  =  (L @ B)[i, j]

    with uniform error ~2e-5 at R=16 (measured for this s distribution; the
    final error is dominated by bf16 input rounding, not the rank).  Row
    sums (softmax denominators) follow from the same factorization:
    den = L @ u with u_k = sum_j B[k, j].  Folding 1/den into A-hat = L/den:

        out = A-hat @ (B @ Z) @ W_C.T

    The N^2 work disappears: the device computes U = B@Z (contraction over
    N, bf16, DMA-bound), folds W_C once into U-tilde = U @ W_C.T, and then
    each 128-row output tile is a single rank-16 matmul  y = A-hat @ U-tilde.
    That matmul cancels ~10x, so it runs in float32r (TF32-like, ~1.6e-4
    measured on HW) — full PE speed at 512-wide moving dim.

    L, B, A-hat are host-computed from s (O(N*R) work, same spirit as the
    host-computed s/SVT/SIB of the direct formulation).

Sharding: output rows split across 8 cores, 1024 each.  Every core streams
the full [Z | B.T] (8192 x 528 bf16) and differs only in its A-hat block.
"""

import numpy as np

N = 8192
D = 512
R = 16                     # Chebyshev rank
NCORES = 8
M = N // NCORES            # 1024 rows per core
P = 128
JT = N // P                # 64 j-tiles
GRP = 4                    # j-tiles per DMA
NG = JT // GRP             # 16 DMA groups
ISUB = M // P              # 8 i-subtiles per core

_CACHE = {}


# --------------------------------------------------------------------------
# Kernel build
# --------------------------------------------------------------------------

def _build():
    import concourse.bass as bass  # noqa: F401
    import concourse.mybir as mybir
    import concourse.tile as tile
    from concourse import bacc
    from concourse.masks import make_identity

    f32 = mybir.dt.float32
    f32r = mybir.dt.float32r
    bf16 = mybir.dt.bfloat16

    nc = bacc.Bacc("TRN2", target_bir_lowering=False, debug=False,
                   num_devices=NCORES)

    # ZB = [Z | B.T] bf16; AHT = (L/den).T f32r for this core's row block;
    # WCT = W_C.T bf16.
    ZB = nc.dram_tensor("ZB", [N, D + R], bf16, kind="ExternalInput").ap()
    AHT = nc.dram_tensor("AHT", [R, M], f32r, kind="ExternalInput").ap()
    WCT = nc.dram_tensor("WCT", [D, D], bf16, kind="ExternalInput").ap()
    Y = nc.dram_tensor("Y", [M, D], bf16, kind="ExternalOutput").ap()

    with tile.TileContext(nc) as tc:
        with (
            tc.tile_pool(name="const", bufs=1) as constp,
            tc.tile_pool(name="zb", bufs=NG) as zbp,
            tc.tile_pool(name="fin", bufs=8) as finp,
            tc.tile_pool(name="psU", bufs=1, space="PSUM") as psU,
            tc.tile_pool(name="psO", bufs=4, space="PSUM") as psO,
        ):
            # ---- stream ZB (the only bulk input) ---------------------------
            # last group is split into single-tile DMAs: per-tile semaphores
            # let the final U matmuls start ~0.6us earlier
            zbs = []
            for g in range(NG - 1):
                zb = zbp.tile([P, GRP, D + R], bf16, tag="zb", name=f"zb{g}")
                nc.sync.dma_start(
                    zb[:],
                    ZB[g * GRP * P:(g + 1) * GRP * P, :].rearrange(
                        "(q p) c -> p q c", p=P))
                zbs.append(zb)
            ztail = []
            for q in range(GRP):
                t = (NG - 1) * GRP + q
                zb1 = zbp.tile([P, 1, D + R], bf16, tag="zb", name=f"zbt{q}")
                nc.sync.dma_start(
                    zb1[:], ZB[t * P:(t + 1) * P, :].rearrange(
                        "(q p) c -> p q c", p=P))
                ztail.append(zb1)

            # small tail-phase inputs, after the ZB stream
            wcb = constp.tile([P, 4, D], bf16)
            nc.sync.dma_start(wcb[:], WCT.rearrange("(dc dd) o -> dd dc o",
                                                    dd=P))
            aht = constp.tile([R, M], f32r)
            nc.sync.dma_start(aht[:], AHT)

            # ---- constants -------------------------------------------------
            # warm the ACT copy-function table during the DMA stream (the
            # first scalar.copy otherwise pays a ~1.3us LoadActFuncSet)
            warm = constp.tile([1, 2], f32)
            nc.vector.memset(warm[:], 0.0)
            nc.scalar.copy(warm[:], warm[:])

            id_b = constp.tile([P, P], bf16)
            make_identity(nc, id_b)

            # Warm the PE HAM clock-gate during the startup DMA window:
            # ~3us of dummy matmul activity lifts the PE to 2.4 GHz before
            # the first real matmul issues.
            for w in range(56):
                wp = psO.tile([P, 64], f32, tag="po", name=f"wp{w}")
                nc.tensor.matmul(wp[:], id_b[:], id_b[:, 0:64],
                                 start=True, stop=True)

            # ---- U.T = (B @ Z).T : 4 PSUM tiles [128 d, 16 k], directly
            # transposed (lhsT = Z d-chunk, rhs = b columns; 16-cycle matmuls
            # that hide under the DMA stream) -------------------------------
            ut_ps4 = [psU.tile([P, R], f32, tag=f"u{dc}", name=f"u{dc}")
                      for dc in range(4)]
            k = 0
            for g in range(NG - 1):
                for q in range(GRP):
                    for dc in range(4):
                        nc.tensor.matmul(
                            ut_ps4[dc][:],
                            zbs[g][:, q, dc * P:(dc + 1) * P],
                            zbs[g][:, q, D:D + R],
                            start=(k == 0), stop=False)
                    k += 1
            for q in range(GRP):
                for dc in range(4):
                    nc.tensor.matmul(
                        ut_ps4[dc][:],
                        ztail[q][:, 0, dc * P:(dc + 1) * P],
                        ztail[q][:, 0, D:D + R],
                        start=False, stop=(q == GRP - 1))

            # ---- fold W_C once: U-tilde = U @ W_C.T ------------------------
            ut = constp.tile([P, 4, R], bf16)
            for dc in range(4):
                nc.vector.tensor_copy(ut[:, dc, :], ut_ps4[dc][:])
            ut_ps = psO.tile([R, D], f32, tag="po", name="utp")
            for dc in range(4):
                nc.tensor.matmul(ut_ps[:], ut[:, dc, :], wcb[:, dc, :],
                                 start=(dc == 0), stop=(dc == 3))
            ut_r = constp.tile([R, D], f32r)
            nc.vector.tensor_copy(ut_r[:, 0:D // 2], ut_ps[:, 0:D // 2])
            nc.vector.tensor_copy(ut_r[:, D // 2:D], ut_ps[:, D // 2:D])

            # ---- per-row-tile: y = A-hat @ U-tilde (f32r), then out --------
            # pairs of row tiles share one output DMA (the per-DMA trigger
            # cost, not bytes, paces the out phase); each PSUM->SBUF copy is
            # split across ACT and DVE so both engines run in parallel
            for gpair in range(ISUB // 2):
                ysb = finp.tile([P, 2, D], bf16, tag="ysb")
                for h in range(2):
                    s = gpair * 2 + h
                    y_ps = psO.tile([P, D], f32, tag="po")
                    nc.tensor.matmul(y_ps[:], aht[:, s * P:(s + 1) * P],
                                     ut_r[:], start=True, stop=True)
                    nc.scalar.copy(ysb[:, h, 0:D // 2], y_ps[:, 0:D // 2])
                    nc.vector.tensor_copy(ysb[:, h, D // 2:D],
                                          y_ps[:, D // 2:D])
                nc.sync.dma_start(
                    Y[gpair * 2 * P:(gpair + 1) * 2 * P, :].rearrange(
                        "(q p) d -> p q d", p=P),
                    ysb[:])

    nc.compile()
    return nc


# --------------------------------------------------------------------------
# Host-side low-rank factor preparation
# --------------------------------------------------------------------------

def make_in_maps(Z, W_C, W_V):
    import ml_dtypes

    Z = np.ascontiguousarray(Z, dtype=np.float32)
    W_C = np.ascontiguousarray(W_C, dtype=np.float32)
    W_V = np.ascontiguousarray(W_V, dtype=np.float32).reshape(D)

    Zb = Z.astype(ml_dtypes.bfloat16)
    # s on the bf16-rounded Z the device also sees
    s = Zb.astype(np.float64) @ W_V.astype(np.float64)

    # Chebyshev nodes covering the realized s range
    lo, hi = s.min() - 1e-3, s.max() + 1e-3
    kk = np.arange(R)
    theta = (2 * kk + 1) * np.pi / (2 * R)
    xk = 0.5 * (lo + hi) + 0.5 * (hi - lo) * np.cos(theta)

    # B[k, j] = g(x_k - s_j), g = exp(sigmoid)
    B = np.exp(1.0 / (1.0 + np.exp(-(xk[:, None] - s[None, :]))))
    Bb = B.astype(ml_dtypes.bfloat16)

    ZBh = np.zeros((N, D + R), dtype=ml_dtypes.bfloat16)
    ZBh[:, :D] = Zb
    ZBh[:, D:] = Bb.T

    # Lagrange basis at the s_i (barycentric form, fp64)
    w = (-1.0) ** kk * np.sin(theta)
    diff = s[:, None] - xk[None, :]
    hit = np.abs(diff) < 1e-300
    with np.errstate(divide="ignore", invalid="ignore"):
        c = w[None, :] / np.where(hit, 1.0, diff)
    L = c / c.sum(axis=1, keepdims=True)
    if hit.any():
        L[hit.any(axis=1)] = hit[hit.any(axis=1)].astype(np.float64)

    # fold softmax denominators (den = L @ u, u from the bf16 B the device
    # actually contracts with)
    u = Bb.astype(np.float64).sum(axis=1)
    den = L @ u
    ahat = L / den[:, None]

    wct = np.ascontiguousarray(W_C.T).astype(ml_dtypes.bfloat16)

    in_maps = []
    for core in range(NCORES):
        ahT = np.ascontiguousarray(
            ahat[core * M:(core + 1) * M].T.astype(np.float32))
        in_maps.append({"ZB": ZBh, "AHT": ahT, "WCT": wct})
    return in_maps


def kernel(Z, W_C, W_V):
    from concourse.bass_utils import run_bass_kernel_spmd

    if "nc" not in _CACHE:
        _CACHE["nc"] = _build()
    nc = _CACHE["nc"]

    in_maps = make_in_maps(Z, W_C, W_V)
    res = run_bass_kernel_spmd(nc, in_maps, core_ids=list(range(NCORES)))
    out = np.empty((N, D), dtype=np.float32)
    for c in range(NCORES):
        out[c * M:(c + 1) * M] = np.asarray(res.results[c]["Y"],
                                            dtype=np.float32)
    return out


# revision 11
# speedup vs baseline: 1.0381x; 1.0124x over previous
"""Trainium2 Bass kernel for nn_ConvectionModule.

Math (reference):
    s = Z @ W_V                                  # [N]
    A = softmax(sigmoid(s_i - s_j), axis=1)      # [N, N]
    out = A @ (Z @ W_C.T)                        # [N, D]

Low-rank formulation used here:
    E[i, j] = g(s_i - s_j),  g(x) = exp(sigmoid(x)),  is a smooth analytic
    kernel of the bounded scalar difference s_i - s_j, hence numerically
    low-rank.  Chebyshev interpolation in the s_i variable on R nodes x_k:

        E[i, j] ~= sum_k L_k(s_i) * g(x_k - s_j)  =  (L @ B)[i, j]

    with uniform error ~2e-5 at R=16 (measured for this s distribution; the
    final error is dominated by bf16 input rounding, not the rank).  Row
    sums (softmax denominators) follow from the same factorization:
    den = L @ u with u_k = sum_j B[k, j].  Folding 1/den into A-hat = L/den:

        out = A-hat @ (B @ Z) @ W_C.T

    The N^2 work disappears: the device computes U = B@Z (contraction over
    N, bf16, DMA-bound), folds W_C once into U-tilde = U @ W_C.T, and then
    each 128-row output tile is a single rank-16 matmul  y = A-hat @ U-tilde.
    That matmul cancels ~10x, so it runs in float32r (TF32-like, ~1.6e-4
    measured on HW) — full PE speed at 512-wide moving dim.

    L, B, A-hat are host-computed from s (O(N*R) work, same spirit as the
    host-computed s/SVT/SIB of the direct formulation).

Sharding: output rows split across 8 cores, 1024 each.  Every core streams
the full [Z | B.T] (8192 x 528 bf16) and differs only in its A-hat block.
"""

import numpy as np

N = 8192
D = 512
R = 16                     # Chebyshev rank
NCORES = 8
M = N // NCORES            # 1024 rows per core
P = 128
JT = N // P                # 64 j-tiles
GRP = 4                    # j-tiles per DMA
NG = JT // GRP             # 16 DMA groups
ISUB = M // P              # 8 i-subtiles per core

_CACHE = {}


# --------------------------------------------------------------------------
# Kernel build
# --------------------------------------------------------------------------

def _build():
    import concourse.bass as bass  # noqa: F401
    import concourse.mybir as mybir
    import concourse.tile as tile
    from concourse import bacc
    from concourse.masks import make_identity

    f32 = mybir.dt.float32
    f32r = mybir.dt.float32r
    bf16 = mybir.dt.bfloat16

    nc = bacc.Bacc("TRN2", target_bir_lowering=False, debug=False,
                   num_devices=NCORES)

    # ZB = [Z | B.T] bf16; AHT = (L/den).T f32r for this core's row block;
    # WCT = W_C.T bf16.
    ZB = nc.dram_tensor("ZB", [N, D + R], bf16, kind="ExternalInput").ap()
    AHT = nc.dram_tensor("AHT", [R, M], f32r, kind="ExternalInput").ap()
    WCT = nc.dram_tensor("WCT", [D, D], bf16, kind="ExternalInput").ap()
    Y = nc.dram_tensor("Y", [M, D], bf16, kind="ExternalOutput").ap()

    with tile.TileContext(nc) as tc:
        with (
            tc.tile_pool(name="const", bufs=1) as constp,
            tc.tile_pool(name="zb", bufs=NG) as zbp,
            tc.tile_pool(name="fin", bufs=8) as finp,
            tc.tile_pool(name="psU", bufs=1, space="PSUM") as psU,
            tc.tile_pool(name="psO", bufs=4, space="PSUM") as psO,
        ):
            # ---- stream ZB (the only bulk input) ---------------------------
            # last group is split into single-tile DMAs: per-tile semaphores
            # let the final U matmuls start ~0.6us earlier
            zbs = []
            for g in range(NG - 1):
                zb = zbp.tile([P, GRP, D + R], bf16, tag="zb", name=f"zb{g}")
                nc.sync.dma_start(
                    zb[:],
                    ZB[g * GRP * P:(g + 1) * GRP * P, :].rearrange(
                        "(q p) c -> p q c", p=P))
                zbs.append(zb)
            ztail = []
            for q in range(GRP):
                t = (NG - 1) * GRP + q
                zb1 = zbp.tile([P, 1, D + R], bf16, tag="zb", name=f"zbt{q}")
                nc.sync.dma_start(
                    zb1[:], ZB[t * P:(t + 1) * P, :].rearrange(
                        "(q p) c -> p q c", p=P))
                ztail.append(zb1)

            # small tail-phase inputs, after the ZB stream; W_C.T comes in
            # four row-block chunks so each U-tilde matmul is gated only on
            # its own chunk
            wcb = constp.tile([P, 4, D], bf16)
            for dc in range(4):
                nc.sync.dma_start(
                    wcb[:, dc, :],
                    WCT[dc * P:(dc + 1) * P, :])
            aht = constp.tile([R, M], f32r)
            nc.sync.dma_start(aht[:], AHT)

            # ---- constants -------------------------------------------------
            # warm the ACT copy-function table during the DMA stream (the
            # first scalar.copy otherwise pays a ~1.3us LoadActFuncSet)
            warm = constp.tile([1, 2], f32)
            nc.vector.memset(warm[:], 0.0)
            nc.scalar.copy(warm[:], warm[:])

            id_b = constp.tile([P, P], bf16)
            make_identity(nc, id_b)

            # Warm the PE HAM clock-gate during the startup DMA window:
            # ~3us of dummy matmul activity lifts the PE to 2.4 GHz before
            # the first real matmul issues.
            for w in range(56):
                wp = psO.tile([P, 64], f32, tag="po", name=f"wp{w}")
                nc.tensor.matmul(wp[:], id_b[:], id_b[:, 0:64],
                                 start=True, stop=True)

            # ---- U.T = (B @ Z).T : 4 PSUM tiles [128 d, 16 k], directly
            # transposed (lhsT = Z d-chunk, rhs = b columns; 16-cycle matmuls
            # that hide under the DMA stream) -------------------------------
            ut_ps4 = [psU.tile([P, R], f32, tag=f"u{dc}", name=f"u{dc}")
                      for dc in range(4)]
            k = 0
            for g in range(NG - 1):
                for q in range(GRP):
                    for dc in range(4):
                        nc.tensor.matmul(
                            ut_ps4[dc][:],
                            zbs[g][:, q, dc * P:(dc + 1) * P],
                            zbs[g][:, q, D:D + R],
                            start=(k == 0), stop=False)
                    k += 1
            for q in range(GRP):
                for dc in range(4):
                    nc.tensor.matmul(
                        ut_ps4[dc][:],
                        ztail[q][:, 0, dc * P:(dc + 1) * P],
                        ztail[q][:, 0, D:D + R],
                        start=False, stop=(q == GRP - 1))

            # ---- fold W_C once: U-tilde = U @ W_C.T ------------------------
            ut = constp.tile([P, 4, R], bf16)
            for dc in range(4):
                if dc % 2 == 0:
                    nc.vector.tensor_copy(ut[:, dc, :], ut_ps4[dc][:])
                else:
                    nc.scalar.copy(ut[:, dc, :], ut_ps4[dc][:])
            ut_ps = psO.tile([R, D], f32, tag="po", name="utp")
            for dc in range(4):
                nc.tensor.matmul(ut_ps[:], ut[:, dc, :], wcb[:, dc, :],
                                 start=(dc == 0), stop=(dc == 3))
            ut_r = constp.tile([R, D], f32r)
            nc.vector.tensor_copy(ut_r[:, 0:D // 2], ut_ps[:, 0:D // 2])
            nc.scalar.copy(ut_r[:, D // 2:D], ut_ps[:, D // 2:D])

            # ---- per-row-tile: y = A-hat @ U-tilde (f32r), then out --------
            # pairs of row tiles share one output DMA (the per-DMA trigger
            # cost, not bytes, paces the out phase); each PSUM->SBUF copy is
            # split across ACT and DVE so both engines run in parallel
            for gpair in range(ISUB // 2):
                ysb = finp.tile([P, 2, D], bf16, tag="ysb")
                for h in range(2):
                    s = gpair * 2 + h
                    if s < 4:
                        y_ps = psO.tile([P, D], f32, tag="po")
                    else:
                        y_ps = psU.tile([P, D], f32, tag=f"u{s - 4}",
                                        name=f"ypsu{s}")
                    nc.tensor.matmul(y_ps[:], aht[:, s * P:(s + 1) * P],
                                     ut_r[:], start=True, stop=True)
                    nc.scalar.copy(ysb[:, h, 0:D // 2], y_ps[:, 0:D // 2])
                    nc.vector.tensor_copy(ysb[:, h, D // 2:D],
                                          y_ps[:, D // 2:D])
                nc.sync.dma_start(
                    Y[gpair * 2 * P:(gpair + 1) * 2 * P, :].rearrange(
                        "(q p) d -> p q d", p=P),
                    ysb[:])

    nc.compile()
    return nc


# --------------------------------------------------------------------------
# Host-side low-rank factor preparation
# --------------------------------------------------------------------------

def make_in_maps(Z, W_C, W_V):
    import ml_dtypes

    Z = np.ascontiguousarray(Z, dtype=np.float32)
    W_C = np.ascontiguousarray(W_C, dtype=np.float32)
    W_V = np.ascontiguousarray(W_V, dtype=np.float32).reshape(D)

    Zb = Z.astype(ml_dtypes.bfloat16)
    # s on the bf16-rounded Z the device also sees
    s = Zb.astype(np.float64) @ W_V.astype(np.float64)

    # Chebyshev nodes covering the realized s range
    lo, hi = s.min() - 1e-3, s.max() + 1e-3
    kk = np.arange(R)
    theta = (2 * kk + 1) * np.pi / (2 * R)
    xk = 0.5 * (lo + hi) + 0.5 * (hi - lo) * np.cos(theta)

    # B[k, j] = g(x_k - s_j), g = exp(sigmoid)
    B = np.exp(1.0 / (1.0 + np.exp(-(xk[:, None] - s[None, :]))))
    Bb = B.astype(ml_dtypes.bfloat16)

    ZBh = np.zeros((N, D + R), dtype=ml_dtypes.bfloat16)
    ZBh[:, :D] = Zb
    ZBh[:, D:] = Bb.T

    # Lagrange basis at the s_i (barycentric form, fp64)
    w = (-1.0) ** kk * np.sin(theta)
    diff = s[:, None] - xk[None, :]
    hit = np.abs(diff) < 1e-300
    with np.errstate(divide="ignore", invalid="ignore"):
        c = w[None, :] / np.where(hit, 1.0, diff)
    L = c / c.sum(axis=1, keepdims=True)
    if hit.any():
        L[hit.any(axis=1)] = hit[hit.any(axis=1)].astype(np.float64)

    # fold softmax denominators (den = L @ u, u from the bf16 B the device
    # actually contracts with)
    u = Bb.astype(np.float64).sum(axis=1)
    den = L @ u
    ahat = L / den[:, None]

    wct = np.ascontiguousarray(W_C.T).astype(ml_dtypes.bfloat16)

    in_maps = []
    for core in range(NCORES):
        ahT = np.ascontiguousarray(
            ahat[core * M:(core + 1) * M].T.astype(np.float32))
        in_maps.append({"ZB": ZBh, "AHT": ahT, "WCT": wct})
    return in_maps


def kernel(Z, W_C, W_V):
    from concourse.bass_utils import run_bass_kernel_spmd

    if "nc" not in _CACHE:
        _CACHE["nc"] = _build()
    nc = _CACHE["nc"]

    in_maps = make_in_maps(Z, W_C, W_V)
    res = run_bass_kernel_spmd(nc, in_maps, core_ids=list(range(NCORES)))
    out = np.empty((N, D), dtype=np.float32)
    for c in range(NCORES):
        out[c * M:(c + 1) * M] = np.asarray(res.results[c]["Y"],
                                            dtype=np.float32)
    return out
